# revision 38
# baseline (speedup 1.0000x reference)
"""Trainium2 Bass kernel for nn_AttentionHead_46660524703758.

Dense software-pipelined single pass: 16 contiguous-row SWDGE cast DMAs
(single-tile -- multi-tile batches make the framework emit a ~25us SWDGE
DRAIN and their descriptor runs stall instruction fetches), x.T via
identity matmuls (drains on VectorE), per-unit [kw|vw] projection
(drains on ScalarE -- engine split breaks a measured 6-7us in-order-queue
convoy), paired K=64 S^T matmuls with lhsT/rhs duplicated onto opposite
partition halves (kT_hi/vT_lo -- lets LDWEIGHTS of one half overlap the
other half's streams; removing it cost ~5us), an additive -30000 causal
mask built right after the 5th DMA issue, one strided exp per 2-bank
chunk, in-loop AV accumulation into 4 persistent PSUM banks, pipelined
final-pair epilogue whose last chunk is split into two 2-tile halves so
the final store starts earlier. No warm-up burst: traces show it never
tripped the HAM clock-gate early enough to pay. XBAR DMA transposes
measured ~1.4us each, serial on the issuing HWDGE engine, and force
SWDGE queue drains when reading SWDGE-written tiles -- not usable.
"""
import sys

if "/opt/trn_rl_repo" not in sys.path:
    sys.path.insert(0, "/opt/trn_rl_repo")

import numpy as np

import concourse.bacc as bacc
import concourse.mybir as mybir
from concourse import tile
from concourse.bass_utils import run_bass_kernel_spmd
from concourse.masks import make_identity

B, N, H, E = 8, 2048, 1024, 64
P = 128
NT = N // P   # 16 n-tiles
NU = 8        # two-tile units
HC = H // P   # 8 h-chunks
DT = mybir.dt.float16
F32 = mybir.dt.float32
EXP_BIAS = -6.0
EXP_SCALE = 0.125  # 1/sqrt(E)
MASK_C = -30000.0
N_WARM = 8

_cached_nc = None


def build_nc():
    nc = bacc.Bacc("TRN2", target_bir_lowering=False, debug=False, num_devices=8)
    x = nc.dram_tensor("x", [N, H], F32, kind="ExternalInput")
    kwt = nc.dram_tensor("kw", [H, E], F32, kind="ExternalInput")
    vwt = nc.dram_tensor("vw", [H, E], F32, kind="ExternalInput")
    out = nc.dram_tensor("out", [N, E], F32, kind="ExternalOutput")

    Exp = mybir.ActivationFunctionType.Exp
    Copy = mybir.ActivationFunctionType.Copy

    with tile.TileContext(nc) as tc:
        with (
            tc.tile_pool(name="const", bufs=1) as const,
            tc.tile_pool(name="big", bufs=1) as big,
            tc.tile_pool(name="xin", bufs=16) as xin,
            tc.tile_pool(name="opool", bufs=2) as opool,
            tc.tile_pool(name="ps", bufs=2, space="PSUM") as psp,
            tc.tile_pool(name="av", bufs=4, space="PSUM") as avp,
        ):
            # --- identity + warm source, then x DMAs immediately ---
            ident_h = const.tile([P, P], DT)
            make_identity(nc, ident_h[:])
            # x input DMAs: 16 single-tile transfers (SWDGE cast fp32->fp16).
            # One contiguous 4KB row per partition — measured ~15% faster than
            # 2-tile units whose strided second row breaks the M2S concat.
            # The mask build rides the gpsimd queue after the 5th issue:
            # ready ~10us earlier for the first S^T pair, without delaying
            # tile 0's descriptor generation.
            mask_t = const.tile([P, P], DT)
            xts = []
            for t in range(NT):
                xt = xin.tile([P, H], DT, tag="xt", name=f"xt{t}")
                nc.gpsimd.dma_start(xt[:], x.ap()[t * P : (t + 1) * P, :])
                xts.append(xt)
                if t == 4:
                    nc.gpsimd.memset(mask_t[:], 0.0)
                    nc.gpsimd.affine_select(
                        out=mask_t[:],
                        in_=mask_t[:],
                        compare_op=mybir.AluOpType.is_ge,
                        fill=MASK_C,
                        base=0,
                        # keep (col - row) >= 0; strict lower triangle gets
                        # MASK_C. The S^T diag block receives mask_t.T.
                        pattern=[[1, P]],
                        channel_multiplier=-1,
                    )

            # --- remaining constants (needed only once pairs start) ---
            bias_t = const.tile([P, 1], F32)
            nc.gpsimd.memset(bias_t[:], EXP_BIAS)
            vnat = big.tile([P, NT * (E + 1)], DT)
            vnat3 = vnat[:].rearrange("p (j c) -> p j c", c=E + 1)
            nc.gpsimd.memset(vnat3[:, :, E : E + 1], 1.0)

            # --- weights via sync HWDGE + DVE pack to fp16 [K|V] ---
            kw_sb = const.tile([P, HC * E], F32)
            vw_sb = const.tile([P, HC * E], F32)
            nc.sync.dma_start(
                kw_sb[:].rearrange("p (c e) -> p c e", c=HC),
                kwt.ap().rearrange("(c p) e -> p c e", p=P),
            )
            nc.sync.dma_start(
                vw_sb[:].rearrange("p (c e) -> p c e", c=HC),
                vwt.ap().rearrange("(c p) e -> p c e", p=P),
            )
            kvw = const.tile([P, HC, P], DT)
            nc.vector.tensor_copy(
                kvw[:, :, 0:E], kw_sb[:].rearrange("p (c e) -> p c e", c=HC)
            )
            nc.vector.tensor_copy(
                kvw[:, :, E:P], vw_sb[:].rearrange("p (c e) -> p c e", c=HC)
            )

            # --- main tensors ---
            xT = big.tile([P, HC * N], DT)  # [h_in_chunk, c*2048 + n]
            xT3 = xT[:].rearrange("p (c n) -> p c n", c=HC)
            kvT = big.tile([P, N], DT)      # rows 0:64 K^T, 64:128 V^T
            vT_lo = big.tile([64, N], DT)   # V^T copy on parts 0:64
            kT_hi = big.tile([P, N], DT)    # rows 64:128 hold a K^T copy
            pT = big.tile([P, NT * N], DT)  # [v_in_tile, i*2048 + k]
            pT3 = pT[:].rearrange("p (i k) -> p i k", i=NT)
            out_sb = big.tile([P, NT * E], F32)

            # --- persistent AV accumulators, one PSUM bank per k-segment ---
            av_tiles = [
                avp.tile([P, 512], F32, tag="av", name=f"av{c}") for c in range(4)
            ]

            av_queue = []  # pending (i, c) AV matmuls

            # ---------- emission helpers ----------
            def emit_tr(t):
                trp = psp.tile([P, 1024], F32, tag="ps", name=f"tr{t}")
                for c in range(HC):
                    nc.tensor.matmul(
                        trp[:, c * P : (c + 1) * P],
                        xts[t][:, c * P : (c + 1) * P],
                        ident_h[:],
                        start=True,
                        stop=True,
                    )
                nc.vector.tensor_copy(
                    xT3[:, :, t * P : (t + 1) * P],
                    trp[:].rearrange("p (c n) -> p c n", c=HC),
                )

            def emit_proju(u):
                c0 = u * 256
                ps_ = psp.tile([P, 1024], F32, tag="ps", name=f"proj{u}")
                for c in range(HC):
                    nc.tensor.matmul(
                        ps_[:, 0:256],
                        kvw[:, c, :],
                        xT3[:, c, c0 : c0 + 256],
                        start=(c == 0),
                        stop=(c == HC - 1),
                    )
                nc.scalar.activation(kvT[:, c0 : c0 + 256], ps_[:, 0:256], Copy)
                nc.sync.dma_start(
                    vT_lo[:, c0 : c0 + 256], kvT[64:128, c0 : c0 + 256]
                )
                nc.scalar.dma_start(
                    kT_hi[64:128, c0 : c0 + 256], kvT[0:64, c0 : c0 + 256]
                )

            def emit_vtru(u):
                vtr = psp.tile([P, 1024], F32, tag="ps", name=f"vtr{u}")
                for jj in range(2):
                    j = 2 * u + jj
                    nc.tensor.matmul(
                        vtr[:, jj * E : (jj + 1) * E],
                        vT_lo[:, j * P : (j + 1) * P],
                        ident_h[0:64, 0:64],
                        start=True,
                        stop=True,
                    )
                nc.scalar.activation(
                    vnat3[:, 2 * u : 2 * u + 2, 0:E],
                    vtr[:, 0 : 2 * E].rearrange("p (j e) -> p j e", j=2),
                    Copy,
                )

            def emit_av(i, c):
                nn = min(512, (i + 1) * P - c * 512)
                nc.tensor.matmul(
                    av_tiles[c][0 : E + 1, 0:nn],
                    vnat3[:, i, :],
                    pT[:, i * N + c * 512 : i * N + c * 512 + nn],
                    start=(i == 4 * c),
                    stop=(i == NT - 1),
                )

            def pe_slack(n):
                for _ in range(min(n, len(av_queue))):
                    i, c = av_queue.pop(0)
                    emit_av(i, c)

            oT_tiles = {}

            def emit_epi_drain(c):
                oT_c = opool.tile([E + 1, 512], DT, tag="oT", name=f"oT{c}")
                nc.vector.tensor_copy(oT_c[:], av_tiles[c][0 : E + 1, :])
                oT_tiles[c] = oT_c

            def emit_epi_pe(c):
                oT_c = oT_tiles[c]
                tr4 = psp.tile([P, 1024], F32, tag="ps", name=f"otr{c}")
                tr4v = tr4[:, 0 : 4 * (E + 1)].rearrange("p (k c) -> p k c", k=4)
                for kt in range(4):
                    nc.tensor.matmul(
                        tr4[:, kt * (E + 1) : (kt + 1) * (E + 1)],
                        oT_c[:, kt * P : (kt + 1) * P],
                        ident_h[0 : E + 1, 0 : E + 1],
                        start=True,
                        stop=True,
                    )
                rec4 = opool.tile([P, 4], F32, tag="rec", name=f"rec{c}")
                nc.vector.reciprocal(
                    rec4[:].rearrange("p (k o) -> p k o", o=1),
                    tr4v[:, :, E : E + 1],
                )
                for kt in range(4):
                    kti = 4 * c + kt
                    if kt % 2 == 0:
                        nc.vector.tensor_scalar_mul(
                            out_sb[:, kti * E : (kti + 1) * E],
                            tr4v[:, kt, 0:E],
                            rec4[:, kt : kt + 1],
                        )
                    else:
                        nc.scalar.activation(
                            out_sb[:, kti * E : (kti + 1) * E],
                            tr4v[:, kt, 0:E],
                            Copy,
                            scale=rec4[:, kt : kt + 1],
                        )
                nc.sync.dma_start(
                    out.ap().rearrange("(t p) e -> p t e", p=P)[:, 4 * c : 4 * c + 4, :],
                    out_sb[:].rearrange("p (t e) -> p t e", t=NT)[
                        :, 4 * c : 4 * c + 4, :
                    ],
                )

            def emit_epi_pe_half(c, h):
                # Tail half-chunk: 2 transposes + recip + 2 scales + a 2-tile
                # out DMA, so the final store starts ~1us sooner.
                oT_c = oT_tiles[c]
                tr4 = psp.tile([P, 1024], F32, tag="ps", name=f"otr{c}_{h}")
                tr4v = tr4[:, 0 : 2 * (E + 1)].rearrange("p (k c) -> p k c", k=2)
                for j in range(2):
                    kt = 2 * h + j
                    nc.tensor.matmul(
                        tr4[:, j * (E + 1) : (j + 1) * (E + 1)],
                        oT_c[:, kt * P : (kt + 1) * P],
                        ident_h[0 : E + 1, 0 : E + 1],
                        start=True,
                        stop=True,
                    )
                rec2 = opool.tile([P, 2], F32, tag="rec", name=f"rec{c}_{h}")
                nc.vector.reciprocal(
                    rec2[:].rearrange("p (k o) -> p k o", o=1),
                    tr4v[:, :, E : E + 1],
                )
                for j in range(2):
                    kti = 4 * c + 2 * h + j
                    if j == 0:
                        nc.vector.tensor_scalar_mul(
                            out_sb[:, kti * E : (kti + 1) * E],
                            tr4v[:, j, 0:E],
                            rec2[:, j : j + 1],
                        )
                    else:
                        nc.scalar.activation(
                            out_sb[:, kti * E : (kti + 1) * E],
                            tr4v[:, j, 0:E],
                            Copy,
                            scale=rec2[:, j : j + 1],
                        )
                t0 = 4 * c + 2 * h
                nc.sync.dma_start(
                    out.ap().rearrange("(t p) e -> p t e", p=P)[:, t0 : t0 + 2, :],
                    out_sb[:].rearrange("p (t e) -> p t e", t=NT)[:, t0 : t0 + 2, :],
                )

            def emit_st_pair(p, final=False):
                a, b = 2 * p, 2 * p + 1
                kend = (b + 1) * P
                nch = (kend + 511) // 512
                for sc in range(nch):
                    ks = sc * 512
                    nn_a = min(512, max(0, (a + 1) * P - ks))
                    nn_b = min(512, kend - ks)
                    st2 = psp.tile([P, 1024], F32, tag="ps", name=f"st{p}_{sc}")
                    diag_a = sc == a // 4
                    diag_b = sc == b // 4
                    if nn_a > 0:
                        nc.tensor.matmul(
                            st2[:, 0:nn_a],
                            vT_lo[:, a * P : (a + 1) * P],
                            kvT[0:64, ks : ks + nn_a],
                            start=True,
                            stop=not diag_a,
                        )
                        if diag_a:
                            da = (a % 4) * P
                            nc.tensor.matmul(
                                st2[:, da : da + P],
                                mask_t[:],
                                ident_h[:],
                                start=False,
                                stop=True,
                            )
                    nc.tensor.matmul(
                        st2[:, 512 : 512 + nn_b],
                        kvT[64:128, b * P : (b + 1) * P],
                        kT_hi[64:128, ks : ks + nn_b],
                        start=True,
                        stop=not diag_b,
                    )
                    if diag_b:
                        db = 512 + (b % 4) * P
                        nc.tensor.matmul(
                            st2[:, db : db + P],
                            mask_t[:],
                            ident_h[:],
                            start=False,
                            stop=True,
                        )
                    if nn_a == 512 and nn_b == 512:
                        nc.scalar.activation(
                            pT3[:, a : b + 1, ks : ks + 512],
                            st2[:].rearrange("p (two k) -> p two k", two=2),
                            Exp,
                            bias=bias_t[:],
                            scale=EXP_SCALE,
                        )
                    else:
                        if nn_a > 0:
                            nc.scalar.activation(
                                pT[:, a * N + ks : a * N + ks + nn_a],
                                st2[:, 0:nn_a],
                                Exp,
                                bias=bias_t[:],
                                scale=EXP_SCALE,
                            )
                        nc.scalar.activation(
                            pT[:, b * N + ks : b * N + ks + nn_b],
                            st2[:, 512 : 512 + nn_b],
                            Exp,
                            bias=bias_t[:],
                            scale=EXP_SCALE,
                        )
                    if not final:
                        pe_slack(2)
                    elif sc >= 1:
                        emit_av(a, sc - 1)
                        emit_av(b, sc - 1)
                        emit_epi_drain(sc - 1)
                        if sc >= 2:
                            emit_epi_pe(sc - 2)
                if final:
                    emit_av(a, nch - 1)
                    emit_av(b, nch - 1)
                    emit_epi_drain(nch - 1)
                    emit_epi_pe(nch - 2)
                    emit_epi_pe_half(nch - 1, 0)
                    emit_epi_pe_half(nch - 1, 1)
                else:
                    av_queue.extend((a, c) for c in range((a + 4) // 4))
                    av_queue.extend((b, c) for c in range((b + 4) // 4))

            # ---------- software-pipelined main loop over units ----------
            emit_tr(0)
            emit_tr(1)
            emit_proju(0)
            for u in range(1, NU):
                emit_tr(2 * u)
                emit_tr(2 * u + 1)
                pe_slack(2)
                emit_st_pair(u - 1)
                emit_vtru(u - 1)
                emit_proju(u)

            # ---------- tail: drain queue, then pair 7 with inline epilogue ---
            pe_slack(len(av_queue))
            emit_vtru(NU - 1)
            emit_st_pair(NU - 1, final=True)

    nc.finalize()
    return nc


def _get_nc():
    global _cached_nc
    if _cached_nc is None:
        _cached_nc = build_nc()
    return _cached_nc


def kernel(input, k, q, v, **extra_bass_kwargs):
    """Full-input entry point: shards batch across 8 cores, gathers output."""
    del q  # reference computes queries but never uses them
    input = np.ascontiguousarray(np.asarray(input, dtype=np.float32))
    k = np.ascontiguousarray(np.asarray(k, dtype=np.float32))
    v = np.ascontiguousarray(np.asarray(v, dtype=np.float32))
    nc = _get_nc()
    in_maps = [{"x": input[b], "kw": k, "vw": v} for b in range(B)]
    res = run_bass_kernel_spmd(
        nc, in_maps, core_ids=list(range(B)), **extra_bass_kwargs
    )
    out = np.stack([r["out"] for r in res.results]).astype(np.float32)
    if extra_bass_kwargs:
        kernel.last_results = res
    return out



# revision 41
# speedup vs baseline: 1.1153x; 1.1153x over previous
"""Trainium2 Bass kernel for nn_AttentionHead_46660524703758.

Dense software-pipelined single pass: 16 contiguous-row SWDGE cast DMAs
(single-tile -- multi-tile batches make the framework emit a ~25us SWDGE
DRAIN and their descriptor runs stall instruction fetches), x.T via
identity matmuls (drains on VectorE), per-unit [kw|vw] projection
(drains on ScalarE -- engine split breaks a measured 6-7us in-order-queue
convoy), paired K=64 S^T matmuls with lhsT/rhs duplicated onto opposite
partition halves (kT_hi/vT_lo -- lets LDWEIGHTS of one half overlap the
other half's streams; removing it cost ~5us), an additive -30000 causal
mask built right after the 5th DMA issue, one strided exp per 2-bank
chunk, in-loop AV accumulation into 4 persistent PSUM banks, pipelined
final-pair epilogue whose last chunk is split into two 2-tile halves so
the final store starts earlier. No warm-up burst: traces show it never
tripped the HAM clock-gate early enough to pay. XBAR DMA transposes
measured ~1.4us each, serial on the issuing HWDGE engine, and force
SWDGE queue drains when reading SWDGE-written tiles -- not usable.
"""
import sys

if "/opt/trn_rl_repo" not in sys.path:
    sys.path.insert(0, "/opt/trn_rl_repo")

import numpy as np

import concourse.bacc as bacc
import concourse.mybir as mybir
from concourse import tile
from concourse.bass_utils import run_bass_kernel_spmd
from concourse.masks import make_identity

B, N, H, E = 8, 2048, 1024, 64
P = 128
NT = N // P   # 16 n-tiles
NU = 8        # two-tile units
HC = H // P   # 8 h-chunks
DT = mybir.dt.float16
F32 = mybir.dt.float32
EXP_BIAS = -6.0
EXP_SCALE = 0.125  # 1/sqrt(E)
MASK_C = -30000.0
N_WARM = 8

_cached_nc = None


def build_nc():
    nc = bacc.Bacc("TRN2", target_bir_lowering=False, debug=False, num_devices=8)
    x = nc.dram_tensor("x", [N, H], F32, kind="ExternalInput")
    kwt = nc.dram_tensor("kw", [H, E], F32, kind="ExternalInput")
    vwt = nc.dram_tensor("vw", [H, E], F32, kind="ExternalInput")
    out = nc.dram_tensor("out", [N, E], F32, kind="ExternalOutput")

    Exp = mybir.ActivationFunctionType.Exp
    Copy = mybir.ActivationFunctionType.Copy

    with tile.TileContext(nc) as tc:
        with (
            tc.tile_pool(name="const", bufs=1) as const,
            tc.tile_pool(name="big", bufs=1) as big,
            tc.tile_pool(name="xin", bufs=16) as xin,
            tc.tile_pool(name="opool", bufs=2) as opool,
            tc.tile_pool(name="ps", bufs=2, space="PSUM") as psp,
            tc.tile_pool(name="av", bufs=4, space="PSUM") as avp,
        ):
            # --- identity + warm source, then x DMAs immediately ---
            ident_h = const.tile([P, P], DT)
            make_identity(nc, ident_h[:])
            # x input DMAs: 16 single-tile transfers (SWDGE cast fp32->fp16).
            # One contiguous 4KB row per partition — measured ~15% faster than
            # 2-tile units whose strided second row breaks the M2S concat.
            # The mask build rides the gpsimd queue after the 5th issue:
            # ready ~10us earlier for the first S^T pair, without delaying
            # tile 0's descriptor generation.
            mask_t = const.tile([P, P], DT)
            xts = []
            for t in range(NT):
                xt = xin.tile([P, H], DT, tag="xt", name=f"xt{t}")
                nc.gpsimd.dma_start(xt[:], x.ap()[t * P : (t + 1) * P, :])
                xts.append(xt)
                if t == 4:
                    nc.gpsimd.memset(mask_t[:], 0.0)
                    nc.gpsimd.affine_select(
                        out=mask_t[:],
                        in_=mask_t[:],
                        compare_op=mybir.AluOpType.is_ge,
                        fill=MASK_C,
                        base=0,
                        # keep (col - row) >= 0; strict lower triangle gets
                        # MASK_C. The S^T diag block receives mask_t.T.
                        pattern=[[1, P]],
                        channel_multiplier=-1,
                    )

            # --- remaining constants (needed only once pairs start) ---
            bias_t = const.tile([P, 1], F32)
            nc.gpsimd.memset(bias_t[:], EXP_BIAS)
            vnat = big.tile([P, NT * (E + 1)], DT)
            vnat3 = vnat[:].rearrange("p (j c) -> p j c", c=E + 1)
            nc.gpsimd.memset(vnat3[:, :, E : E + 1], 1.0)

            # --- weights via sync HWDGE + DVE pack to fp16 [K|V] ---
            kw_sb = const.tile([P, HC * E], F32)
            vw_sb = const.tile([P, HC * E], F32)
            nc.sync.dma_start(
                kw_sb[:].rearrange("p (c e) -> p c e", c=HC),
                kwt.ap().rearrange("(c p) e -> p c e", p=P),
            )
            nc.sync.dma_start(
                vw_sb[:].rearrange("p (c e) -> p c e", c=HC),
                vwt.ap().rearrange("(c p) e -> p c e", p=P),
            )
            kvw = const.tile([P, HC, P], DT)
            nc.vector.tensor_copy(
                kvw[:, :, 0:E], kw_sb[:].rearrange("p (c e) -> p c e", c=HC)
            )
            nc.vector.tensor_copy(
                kvw[:, :, E:P], vw_sb[:].rearrange("p (c e) -> p c e", c=HC)
            )

            # --- main tensors ---
            xT = big.tile([P, HC * N], DT)  # [h_in_chunk, c*2048 + n]
            xT3 = xT[:].rearrange("p (c n) -> p c n", c=HC)
            kvT = big.tile([P, N], DT)      # rows 0:64 K^T, 64:128 V^T
            vT_lo = big.tile([64, N], DT)   # V^T copy on parts 0:64
            kT_hi = big.tile([P, N], DT)    # rows 64:128 hold a K^T copy
            pT = big.tile([P, NT * N], DT)  # [v_in_tile, i*2048 + k]
            pT3 = pT[:].rearrange("p (i k) -> p i k", i=NT)
            out_sb = big.tile([P, NT * E], F32)

            # --- persistent AV accumulators, one PSUM bank per k-segment ---
            av_tiles = [
                avp.tile([P, 512], F32, tag="av", name=f"av{c}") for c in range(4)
            ]

            av_queue = []  # pending (i, c) AV matmuls

            # ---------- emission helpers ----------
            def emit_tr(t):
                trp = psp.tile([P, 1024], F32, tag="ps", name=f"tr{t}")
                for c in range(HC):
                    nc.tensor.matmul(
                        trp[:, c * P : (c + 1) * P],
                        xts[t][:, c * P : (c + 1) * P],
                        ident_h[:],
                        start=True,
                        stop=True,
                    )
                nc.vector.tensor_copy(
                    xT3[:, :, t * P : (t + 1) * P],
                    trp[:].rearrange("p (c n) -> p c n", c=HC),
                )

            def emit_proju(u):
                c0 = u * 256
                ps_ = psp.tile([P, 1024], F32, tag="ps", name=f"proj{u}")
                for c in range(HC):
                    nc.tensor.matmul(
                        ps_[:, 0:256],
                        kvw[:, c, :],
                        xT3[:, c, c0 : c0 + 256],
                        start=(c == 0),
                        stop=(c == HC - 1),
                    )
                nc.scalar.activation(kvT[:, c0 : c0 + 256], ps_[:, 0:256], Copy)
                nc.sync.dma_start(
                    vT_lo[:, c0 : c0 + 256], kvT[64:128, c0 : c0 + 256]
                )
                nc.scalar.dma_start(
                    kT_hi[64:128, c0 : c0 + 256], kvT[0:64, c0 : c0 + 256]
                )

            def emit_vtru(u):
                vtr = psp.tile([P, 1024], F32, tag="ps", name=f"vtr{u}")
                for jj in range(2):
                    j = 2 * u + jj
                    nc.tensor.matmul(
                        vtr[:, jj * E : (jj + 1) * E],
                        vT_lo[:, j * P : (j + 1) * P],
                        ident_h[0:64, 0:64],
                        start=True,
                        stop=True,
                    )
                nc.scalar.activation(
                    vnat3[:, 2 * u : 2 * u + 2, 0:E],
                    vtr[:, 0 : 2 * E].rearrange("p (j e) -> p j e", j=2),
                    Copy,
                )

            def emit_av(i, c):
                nn = min(512, (i + 1) * P - c * 512)
                nc.tensor.matmul(
                    av_tiles[c][0 : E + 1, 0:nn],
                    vnat3[:, i, :],
                    pT[:, i * N + c * 512 : i * N + c * 512 + nn],
                    start=(i == 4 * c),
                    stop=(i == NT - 1),
                )

            def pe_slack(n):
                for _ in range(min(n, len(av_queue))):
                    i, c = av_queue.pop(0)
                    emit_av(i, c)

            oT_tiles = {}

            def emit_epi_drain(c):
                oT_c = opool.tile([E + 1, 512], DT, tag="oT", name=f"oT{c}")
                nc.vector.tensor_copy(oT_c[:], av_tiles[c][0 : E + 1, :])
                oT_tiles[c] = oT_c

            def emit_epi_pe(c):
                oT_c = oT_tiles[c]
                tr4 = psp.tile([P, 1024], F32, tag="ps", name=f"otr{c}")
                tr4v = tr4[:, 0 : 4 * (E + 1)].rearrange("p (k c) -> p k c", k=4)
                for kt in range(4):
                    nc.tensor.matmul(
                        tr4[:, kt * (E + 1) : (kt + 1) * (E + 1)],
                        oT_c[:, kt * P : (kt + 1) * P],
                        ident_h[0 : E + 1, 0 : E + 1],
                        start=True,
                        stop=True,
                    )
                rec4 = opool.tile([P, 4], F32, tag="rec", name=f"rec{c}")
                nc.vector.reciprocal(
                    rec4[:].rearrange("p (k o) -> p k o", o=1),
                    tr4v[:, :, E : E + 1],
                )
                for kt in range(4):
                    kti = 4 * c + kt
                    if kt % 2 == 0:
                        nc.vector.tensor_scalar_mul(
                            out_sb[:, kti * E : (kti + 1) * E],
                            tr4v[:, kt, 0:E],
                            rec4[:, kt : kt + 1],
                        )
                    else:
                        nc.scalar.activation(
                            out_sb[:, kti * E : (kti + 1) * E],
                            tr4v[:, kt, 0:E],
                            Copy,
                            scale=rec4[:, kt : kt + 1],
                        )
                nc.sync.dma_start(
                    out.ap().rearrange("(t p) e -> p t e", p=P)[:, 4 * c : 4 * c + 4, :],
                    out_sb[:].rearrange("p (t e) -> p t e", t=NT)[
                        :, 4 * c : 4 * c + 4, :
                    ],
                )

            def emit_epi_pe_half(c, h):
                # Tail half-chunk: 2 transposes + recip + 2 scales + a 2-tile
                # out DMA, so the final store starts ~1us sooner.
                oT_c = oT_tiles[c]
                tr4 = psp.tile([P, 1024], F32, tag="ps", name=f"otr{c}_{h}")
                tr4v = tr4[:, 0 : 2 * (E + 1)].rearrange("p (k c) -> p k c", k=2)
                for j in range(2):
                    kt = 2 * h + j
                    nc.tensor.matmul(
                        tr4[:, j * (E + 1) : (j + 1) * (E + 1)],
                        oT_c[:, kt * P : (kt + 1) * P],
                        ident_h[0 : E + 1, 0 : E + 1],
                        start=True,
                        stop=True,
                    )
                rec2 = opool.tile([P, 2], F32, tag="rec", name=f"rec{c}_{h}")
                nc.vector.reciprocal(
                    rec2[:].rearrange("p (k o) -> p k o", o=1),
                    tr4v[:, :, E : E + 1],
                )
                for j in range(2):
                    kti = 4 * c + 2 * h + j
                    if j == 0:
                        nc.vector.tensor_scalar_mul(
                            out_sb[:, kti * E : (kti + 1) * E],
                            tr4v[:, j, 0:E],
                            rec2[:, j : j + 1],
                        )
                    else:
                        nc.scalar.activation(
                            out_sb[:, kti * E : (kti + 1) * E],
                            tr4v[:, j, 0:E],
                            Copy,
                            scale=rec2[:, j : j + 1],
                        )
                t0 = 4 * c + 2 * h
                nc.sync.dma_start(
                    out.ap().rearrange("(t p) e -> p t e", p=P)[:, t0 : t0 + 2, :],
                    out_sb[:].rearrange("p (t e) -> p t e", t=NT)[:, t0 : t0 + 2, :],
                )

            def emit_st_pair(p, final=False):
                a, b = 2 * p, 2 * p + 1
                kend = (b + 1) * P
                nch = (kend + 511) // 512
                for sc in range(nch):
                    ks = sc * 512
                    nn_a = min(512, max(0, (a + 1) * P - ks))
                    nn_b = min(512, kend - ks)
                    st2 = psp.tile([P, 1024], F32, tag="ps", name=f"st{p}_{sc}")
                    diag_a = sc == a // 4
                    diag_b = sc == b // 4
                    if nn_a > 0:
                        nc.tensor.matmul(
                            st2[:, 0:nn_a],
                            vT_lo[:, a * P : (a + 1) * P],
                            kvT[0:64, ks : ks + nn_a],
                            start=True,
                            stop=not diag_a,
                        )
                        if diag_a:
                            da = (a % 4) * P
                            nc.tensor.matmul(
                                st2[:, da : da + P],
                                mask_t[:],
                                ident_h[:],
                                start=False,
                                stop=True,
                            )
                    nc.tensor.matmul(
                        st2[:, 512 : 512 + nn_b],
                        kvT[64:128, b * P : (b + 1) * P],
                        kT_hi[64:128, ks : ks + nn_b],
                        start=True,
                        stop=not diag_b,
                    )
                    if diag_b:
                        db = 512 + (b % 4) * P
                        nc.tensor.matmul(
                            st2[:, db : db + P],
                            mask_t[:],
                            ident_h[:],
                            start=False,
                            stop=True,
                        )
                    if nn_a == 512 and nn_b == 512:
                        nc.scalar.activation(
                            pT3[:, a : b + 1, ks : ks + 512],
                            st2[:].rearrange("p (two k) -> p two k", two=2),
                            Exp,
                            bias=bias_t[:],
                            scale=EXP_SCALE,
                        )
                    else:
                        if nn_a > 0:
                            nc.scalar.activation(
                                pT[:, a * N + ks : a * N + ks + nn_a],
                                st2[:, 0:nn_a],
                                Exp,
                                bias=bias_t[:],
                                scale=EXP_SCALE,
                            )
                        nc.scalar.activation(
                            pT[:, b * N + ks : b * N + ks + nn_b],
                            st2[:, 512 : 512 + nn_b],
                            Exp,
                            bias=bias_t[:],
                            scale=EXP_SCALE,
                        )
                    if not final:
                        pe_slack(2)
                    elif sc >= 1:
                        emit_av(a, sc - 1)
                        emit_av(b, sc - 1)
                        emit_epi_drain(sc - 1)
                        if sc >= 2:
                            emit_epi_pe(sc - 2)
                if final:
                    emit_av(a, nch - 1)
                    emit_av(b, nch - 1)
                    emit_epi_drain(nch - 1)
                    emit_epi_pe(nch - 2)
                    emit_epi_pe_half(nch - 1, 0)
                    emit_epi_pe_half(nch - 1, 1)
                else:
                    av_queue.extend((a, c) for c in range((a + 4) // 4))
                    av_queue.extend((b, c) for c in range((b + 4) // 4))

            # ---------- software-pipelined main loop over units ----------
            emit_tr(0)
            emit_tr(1)
            emit_proju(0)
            for u in range(1, NU):
                emit_tr(2 * u)
                emit_tr(2 * u + 1)
                pe_slack(2)
                emit_st_pair(u - 1)
                emit_vtru(u - 1)
                emit_proju(u)

            # ---------- tail: drain queue, then pair 7 with inline epilogue ---
            pe_slack(len(av_queue))
            emit_vtru(NU - 1)
            emit_st_pair(NU - 1, final=True)

    nc.finalize()
    return nc


def _get_nc():
    global _cached_nc
    if _cached_nc is None:
        _cached_nc = build_nc()
    return _cached_nc


def kernel(input, k, q, v, **extra_bass_kwargs):
    """Full-input entry point: shards batch across 8 cores, gathers output."""
    del q  # reference computes queries but never uses them
    input = np.ascontiguousarray(np.asarray(input, dtype=np.float32))
    k = np.ascontiguousarray(np.asarray(k, dtype=np.float32))
    v = np.ascontiguousarray(np.asarray(v, dtype=np.float32))
    nc = _get_nc()
    in_maps = [{"x": input[b], "kw": k, "vw": v} for b in range(B)]
    res = run_bass_kernel_spmd(
        nc, in_maps, core_ids=list(range(B)), **extra_bass_kwargs
    )
    out = np.stack([r["out"] for r in res.results]).astype(np.float32)
    if extra_bass_kwargs:
        kernel.last_results = res
    return out



# revision 49
# speedup vs baseline: 1.1706x; 1.0496x over previous
"""Trainium2 Bass kernel for nn_AttentionHead_46660524703758.

Dense software-pipelined single pass: 16 contiguous-row SWDGE cast DMAs
(single-tile -- multi-tile batches make the framework emit a ~25us SWDGE
DRAIN and their descriptor runs stall instruction fetches), x.T via
identity matmuls (drains on VectorE), per-unit [kw|vw] projection
(drains on ScalarE -- engine split breaks a measured 6-7us in-order-queue
convoy), paired K=64 S^T matmuls with lhsT/rhs duplicated onto opposite
partition halves (kT_hi/vT_lo -- lets LDWEIGHTS of one half overlap the
other half's streams; removing it cost ~5us), an additive -30000 causal
mask built right after the 5th DMA issue, one strided exp per 2-bank
chunk, in-loop AV accumulation into 4 persistent PSUM banks, pipelined
final-pair epilogue whose last chunk is split into two 2-tile halves so
the final store starts earlier. No warm-up burst: traces show it never
tripped the HAM clock-gate early enough to pay. XBAR DMA transposes
measured ~1.4us each, serial on the issuing HWDGE engine, and force
SWDGE queue drains when reading SWDGE-written tiles -- not usable.
"""
import sys

if "/opt/trn_rl_repo" not in sys.path:
    sys.path.insert(0, "/opt/trn_rl_repo")

import numpy as np

import concourse.bacc as bacc
import concourse.mybir as mybir
from concourse import tile
from concourse.bass_utils import run_bass_kernel_spmd
from concourse.masks import make_identity

B, N, H, E = 8, 2048, 1024, 64
P = 128
NT = N // P   # 16 n-tiles
NU = 8        # two-tile units
HC = H // P   # 8 h-chunks
DT = mybir.dt.float16
F32 = mybir.dt.float32
EXP_BIAS = -6.0
EXP_SCALE = 0.125  # 1/sqrt(E)
MASK_C = -30000.0
N_WARM = 8

_cached_nc = None


def build_nc():
    nc = bacc.Bacc("TRN2", target_bir_lowering=False, debug=False, num_devices=8)
    x = nc.dram_tensor("x", [N, H], F32, kind="ExternalInput")
    kwt = nc.dram_tensor("kw", [H, E], F32, kind="ExternalInput")
    vwt = nc.dram_tensor("vw", [H, E], F32, kind="ExternalInput")
    out = nc.dram_tensor("out", [N, E], F32, kind="ExternalOutput")

    Exp = mybir.ActivationFunctionType.Exp
    Copy = mybir.ActivationFunctionType.Copy

    with tile.TileContext(nc) as tc:
        with (
            tc.tile_pool(name="const", bufs=1) as const,
            tc.tile_pool(name="big", bufs=1) as big,
            tc.tile_pool(name="xin", bufs=16) as xin,
            tc.tile_pool(name="opool", bufs=2) as opool,
            tc.tile_pool(name="ps", bufs=2, space="PSUM") as psp,
            tc.tile_pool(name="av", bufs=4, space="PSUM") as avp,
        ):
            # --- identity + warm source, then x DMAs immediately ---
            ident_h = const.tile([P, P], DT)
            make_identity(nc, ident_h[:])
            # x input DMAs: 16 single-tile transfers (SWDGE cast fp32->fp16).
            # One contiguous 4KB row per partition — measured ~15% faster than
            # 2-tile units whose strided second row breaks the M2S concat.
            xts = []
            for t in range(NT):
                xt = xin.tile([P, H], DT, tag="xt", name=f"xt{t}")
                nc.gpsimd.dma_start(xt[:], x.ap()[t * P : (t + 1) * P, :])
                xts.append(xt)

            # --- remaining constants (needed only once pairs start) ---
            bias_t = const.tile([P, 1], F32)
            nc.gpsimd.memset(bias_t[:], EXP_BIAS)
            vnat = big.tile([P, NT * (E + 1)], DT)
            vnat3 = vnat[:].rearrange("p (j c) -> p j c", c=E + 1)
            nc.gpsimd.memset(vnat3[:, :, E : E + 1], 1.0)

            # --- weights via sync HWDGE + DVE pack to fp16 [K|V] ---
            kw_sb = const.tile([P, HC * E], F32)
            vw_sb = const.tile([P, HC * E], F32)
            nc.sync.dma_start(
                kw_sb[:].rearrange("p (c e) -> p c e", c=HC),
                kwt.ap().rearrange("(c p) e -> p c e", p=P),
            )
            nc.sync.dma_start(
                vw_sb[:].rearrange("p (c e) -> p c e", c=HC),
                vwt.ap().rearrange("(c p) e -> p c e", p=P),
            )
            kvw = const.tile([P, HC, P], DT)
            nc.vector.tensor_copy(
                kvw[:, :, 0:E], kw_sb[:].rearrange("p (c e) -> p c e", c=HC)
            )
            nc.vector.tensor_copy(
                kvw[:, :, E:P], vw_sb[:].rearrange("p (c e) -> p c e", c=HC)
            )

            # --- main tensors ---
            xT = big.tile([P, HC * N], DT)  # [h_in_chunk, c*2048 + n]
            xT3 = xT[:].rearrange("p (c n) -> p c n", c=HC)
            kvT = big.tile([P, N], DT)      # rows 0:64 K^T, 64:128 V^T
            vT_lo = big.tile([64, N], DT)   # V^T copy on parts 0:64
            kT_hi = big.tile([P, N], DT)    # rows 64:128 hold a K^T copy
            pT = big.tile([P, NT * N], DT)  # [v_in_tile, i*2048 + k]
            pT3 = pT[:].rearrange("p (i k) -> p i k", i=NT)
            out_sb = big.tile([P, NT * E], F32)

            # --- persistent AV accumulators, one PSUM bank per k-segment ---
            av_tiles = [
                avp.tile([P, 512], F32, tag="av", name=f"av{c}") for c in range(4)
            ]

            av_queue = []  # pending (i, c) AV matmuls

            # ---------- emission helpers ----------
            def emit_tr(t):
                trp = psp.tile([P, 1024], F32, tag="ps", name=f"tr{t}")
                for c in range(HC):
                    nc.tensor.matmul(
                        trp[:, c * P : (c + 1) * P],
                        xts[t][:, c * P : (c + 1) * P],
                        ident_h[:],
                        start=True,
                        stop=True,
                    )
                trp3 = trp[:].rearrange("p (c n) -> p c n", c=HC)
                if t <= 5:
                    # Load phase: Scalar idles until the first exps (~21us),
                    # so split the drain DVE||Scalar - halves the 1.2us
                    # latency gating proj(0..2) -> st(0..2).
                    nc.vector.tensor_copy(
                        xT3[:, 0:4, t * P : (t + 1) * P], trp3[:, 0:4, :]
                    )
                    nc.scalar.activation(
                        xT3[:, 4:8, t * P : (t + 1) * P], trp3[:, 4:8, :], Copy
                    )
                else:
                    nc.vector.tensor_copy(
                        xT3[:, :, t * P : (t + 1) * P], trp3
                    )

            def emit_proju(u):
                c0 = u * 256
                ps_ = psp.tile([P, 1024], F32, tag="ps", name=f"proj{u}")
                for c in range(HC):
                    nc.tensor.matmul(
                        ps_[:, 0:256],
                        kvw[:, c, :],
                        xT3[:, c, c0 : c0 + 256],
                        start=(c == 0),
                        stop=(c == HC - 1),
                    )
                nc.scalar.activation(kvT[:, c0 : c0 + 256], ps_[:, 0:256], Copy)
                nc.sync.dma_start(
                    vT_lo[:, c0 : c0 + 256], kvT[64:128, c0 : c0 + 256]
                )
                nc.scalar.dma_start(
                    kT_hi[64:128, c0 : c0 + 256], kvT[0:64, c0 : c0 + 256]
                )

            def emit_vtru(u):
                vtr = psp.tile([P, 1024], F32, tag="ps", name=f"vtr{u}")
                for jj in range(2):
                    j = 2 * u + jj
                    nc.tensor.matmul(
                        vtr[:, jj * E : (jj + 1) * E],
                        vT_lo[:, j * P : (j + 1) * P],
                        ident_h[0:64, 0:64],
                        start=True,
                        stop=True,
                    )
                nc.scalar.activation(
                    vnat3[:, 2 * u : 2 * u + 2, 0:E],
                    vtr[:, 0 : 2 * E].rearrange("p (j e) -> p j e", j=2),
                    Copy,
                )

            def emit_av(i, c):
                nn = min(512, (i + 1) * P - c * 512)
                nc.tensor.matmul(
                    av_tiles[c][0 : E + 1, 0:nn],
                    vnat3[:, i, :],
                    pT[:, i * N + c * 512 : i * N + c * 512 + nn],
                    start=(i == 4 * c),
                    stop=(i == NT - 1),
                )

            def pe_slack(n):
                for _ in range(min(n, len(av_queue))):
                    i, c = av_queue.pop(0)
                    emit_av(i, c)

            oT_tiles = {}

            def emit_epi_drain(c):
                oT_c = opool.tile([E + 1, 512], DT, tag="oT", name=f"oT{c}")
                nc.vector.tensor_copy(oT_c[:], av_tiles[c][0 : E + 1, :])
                oT_tiles[c] = oT_c

            def emit_epi_pe(c):
                oT_c = oT_tiles[c]
                tr4 = psp.tile([P, 1024], F32, tag="ps", name=f"otr{c}")
                tr4v = tr4[:, 0 : 4 * (E + 1)].rearrange("p (k c) -> p k c", k=4)
                for kt in range(4):
                    nc.tensor.matmul(
                        tr4[:, kt * (E + 1) : (kt + 1) * (E + 1)],
                        oT_c[:, kt * P : (kt + 1) * P],
                        ident_h[0 : E + 1, 0 : E + 1],
                        start=True,
                        stop=True,
                    )
                rec4 = opool.tile([P, 4], F32, tag="rec", name=f"rec{c}")
                nc.vector.reciprocal(
                    rec4[:].rearrange("p (k o) -> p k o", o=1),
                    tr4v[:, :, E : E + 1],
                )
                for kt in range(4):
                    kti = 4 * c + kt
                    if kt % 2 == 0:
                        nc.vector.tensor_scalar_mul(
                            out_sb[:, kti * E : (kti + 1) * E],
                            tr4v[:, kt, 0:E],
                            rec4[:, kt : kt + 1],
                        )
                    else:
                        nc.scalar.activation(
                            out_sb[:, kti * E : (kti + 1) * E],
                            tr4v[:, kt, 0:E],
                            Copy,
                            scale=rec4[:, kt : kt + 1],
                        )
                nc.sync.dma_start(
                    out.ap().rearrange("(t p) e -> p t e", p=P)[:, 4 * c : 4 * c + 4, :],
                    out_sb[:].rearrange("p (t e) -> p t e", t=NT)[
                        :, 4 * c : 4 * c + 4, :
                    ],
                )

            def emit_epi_pe_half(c, h):
                # Tail half-chunk: 2 transposes + recip + 2 scales + a 2-tile
                # out DMA, so the final store starts ~1us sooner.
                oT_c = oT_tiles[c]
                tr4 = psp.tile([P, 1024], F32, tag="ps", name=f"otr{c}_{h}")
                tr4v = tr4[:, 0 : 2 * (E + 1)].rearrange("p (k c) -> p k c", k=2)
                for j in range(2):
                    kt = 2 * h + j
                    nc.tensor.matmul(
                        tr4[:, j * (E + 1) : (j + 1) * (E + 1)],
                        oT_c[:, kt * P : (kt + 1) * P],
                        ident_h[0 : E + 1, 0 : E + 1],
                        start=True,
                        stop=True,
                    )
                rec2 = opool.tile([P, 2], F32, tag="rec", name=f"rec{c}_{h}")
                nc.vector.reciprocal(
                    rec2[:].rearrange("p (k o) -> p k o", o=1),
                    tr4v[:, :, E : E + 1],
                )
                for j in range(2):
                    kti = 4 * c + 2 * h + j
                    if j == 0:
                        nc.vector.tensor_scalar_mul(
                            out_sb[:, kti * E : (kti + 1) * E],
                            tr4v[:, j, 0:E],
                            rec2[:, j : j + 1],
                        )
                    else:
                        nc.scalar.activation(
                            out_sb[:, kti * E : (kti + 1) * E],
                            tr4v[:, j, 0:E],
                            Copy,
                            scale=rec2[:, j : j + 1],
                        )
                t0 = 4 * c + 2 * h
                nc.sync.dma_start(
                    out.ap().rearrange("(t p) e -> p t e", p=P)[:, t0 : t0 + 2, :],
                    out_sb[:].rearrange("p (t e) -> p t e", t=NT)[:, t0 : t0 + 2, :],
                )

            def emit_st_pair(p, final=False):
                a, b = 2 * p, 2 * p + 1
                kend = (b + 1) * P
                nch = (kend + 511) // 512
                for sc in range(nch):
                    ks = sc * 512
                    nn_a = min(512, max(0, (a + 1) * P - ks))
                    nn_b = min(512, kend - ks)
                    st2 = psp.tile([P, 1024], F32, tag="ps", name=f"st{p}_{sc}")
                    diag_a = sc == a // 4
                    diag_b = sc == b // 4
                    if nn_a > 0:
                        nc.tensor.matmul(
                            st2[:, 0:nn_a],
                            vT_lo[:, a * P : (a + 1) * P],
                            kvT[0:64, ks : ks + nn_a],
                            start=True,
                            stop=True,
                        )
                    nc.tensor.matmul(
                        st2[:, 512 : 512 + nn_b],
                        kvT[64:128, b * P : (b + 1) * P],
                        kT_hi[64:128, ks : ks + nn_b],
                        start=True,
                        stop=True,
                    )
                    if nn_a == 512 and nn_b == 512:
                        nc.scalar.activation(
                            pT3[:, a : b + 1, ks : ks + 512],
                            st2[:].rearrange("p (two k) -> p two k", two=2),
                            Exp,
                            bias=bias_t[:],
                            scale=EXP_SCALE,
                        )
                    else:
                        if nn_a > 0:
                            nc.scalar.activation(
                                pT[:, a * N + ks : a * N + ks + nn_a],
                                st2[:, 0:nn_a],
                                Exp,
                                bias=bias_t[:],
                                scale=EXP_SCALE,
                            )
                        nc.scalar.activation(
                            pT[:, b * N + ks : b * N + ks + nn_b],
                            st2[:, 512 : 512 + nn_b],
                            Exp,
                            bias=bias_t[:],
                            scale=EXP_SCALE,
                        )
                    for diag, i in ((diag_a, a), (diag_b, b)):
                        if diag:
                            # Causal mask: zero exp values where k > v in the
                            # 128x128 diagonal block. Runs on GpSimd (idle all
                            # mid-kernel; pT is SBUF) - replaces 16 PE mask
                            # matmuls + LDWEIGHTS (~3us of cold-clock PE).
                            nc.gpsimd.affine_select(
                                out=pT3[:, i, i * P : (i + 1) * P],
                                in_=pT3[:, i, i * P : (i + 1) * P],
                                compare_op=mybir.AluOpType.is_ge,
                                fill=0.0,
                                base=0,
                                pattern=[[-1, P]],
                                channel_multiplier=1,
                            )
                    if not final:
                        pe_slack(2)
                    elif sc >= 1:
                        emit_av(a, sc - 1)
                        emit_av(b, sc - 1)
                        emit_epi_drain(sc - 1)
                        if sc >= 2:
                            emit_epi_pe(sc - 2)
                if final:
                    emit_av(a, nch - 1)
                    emit_av(b, nch - 1)
                    emit_epi_drain(nch - 1)
                    emit_epi_pe(nch - 2)
                    emit_epi_pe_half(nch - 1, 0)
                    emit_epi_pe_half(nch - 1, 1)
                else:
                    av_queue.extend((a, c) for c in range((a + 4) // 4))
                    av_queue.extend((b, c) for c in range((b + 4) // 4))

            # ---------- software-pipelined main loop over units ----------
            emit_tr(0)
            emit_tr(1)
            emit_proju(0)
            for u in range(1, NU):
                emit_tr(2 * u)
                emit_tr(2 * u + 1)
                pe_slack(2)
                emit_st_pair(u - 1)
                emit_vtru(u - 1)
                emit_proju(u)

            # ---------- tail: drain queue, then pair 7 with inline epilogue ---
            pe_slack(len(av_queue))
            emit_vtru(NU - 1)
            emit_st_pair(NU - 1, final=True)

    nc.finalize()
    return nc


def _get_nc():
    global _cached_nc
    if _cached_nc is None:
        _cached_nc = build_nc()
    return _cached_nc


def kernel(input, k, q, v, **extra_bass_kwargs):
    """Full-input entry point: shards batch across 8 cores, gathers output."""
    del q  # reference computes queries but never uses them
    input = np.ascontiguousarray(np.asarray(input, dtype=np.float32))
    k = np.ascontiguousarray(np.asarray(k, dtype=np.float32))
    v = np.ascontiguousarray(np.asarray(v, dtype=np.float32))
    nc = _get_nc()
    in_maps = [{"x": input[b], "kw": k, "vw": v} for b in range(B)]
    res = run_bass_kernel_spmd(
        nc, in_maps, core_ids=list(range(B)), **extra_bass_kwargs
    )
    out = np.stack([r["out"] for r in res.results]).astype(np.float32)
    if extra_bass_kwargs:
        kernel.last_results = res
    return out

